# revision 85
# baseline (speedup 1.0000x reference)
"""Trainium2 Bass kernel for nn_Encoder_Cross (dense transformer encoder).

Data-parallel over batch: 8 batches -> 8 NeuronCores, weights replicated.

v3 restructure (vs v2 baseline):
  - Each phase split into two loops so the ACT engine uses ONE activation
    table per loop (A/C: Exp, B/D: Gelu) -> activation-table reloads drop
    from ~193 to ~5.
  - LN rstd computed without ACT Sqrt: quadratic seed + 1 Newton step on
    DVE/GPSIMD (var is always near 1 for LN1..LN4; LN0 keeps Sqrt once).
  - Softmax mask folded into the Exp bias (per-partition log-mask).
  - c4 (log-count bias) injected into the score psum by an f32 matmul.
  - Phase-2 feature-major fp8 activations (x2T8) are produced in phase 1,
    removing phase-2 DMA transposes; x3T8 likewise produced in loop C.
  - Elementwise work spread across ACT / DVE / GPSIMD(Pool, SBUF-only).
"""
import sys

sys.path.insert(0, "/opt/trn_rl_repo")

import numpy as np
import ml_dtypes

import concourse.bass as bass
import concourse.mybir as mybir
import concourse.tile as tile
from concourse import bacc
from concourse.bass_utils import run_bass_kernel_spmd
from concourse.masks import make_identity

F32 = mybir.dt.float32
BF16 = mybir.dt.bfloat16
FP8 = mybir.dt.float8e4

B, V, P, D = 8, 32, 256, 512
PERIOD = 16
S = P // PERIOD          # 16 pooled slots per variable
PARTIAL = 2
PQ = P // PARTIAL        # 128 query positions in cross attention
N_REL = 8
LN_EPS = 1e-5
SCALE = 1.0 / float(np.sqrt(D))
H2 = 2 * D
N_CORES = 8

SM = 8.0                 # M matrix fp8 scale
SACT = 8.0               # q2/k2 fp8 scale (activations std ~0.45)
S_O2 = 1.0 / 64.0        # o2 psum -> fp8 scale

# weight fp8 scales, set by _prep_weights (host absmax -> power of 2)
WSCALE = {}

DEBUG = False
_DBG = {}

# engine choice per op: 'act' | 'dve' | 'pool' (pool = SBUF-only!)
ECH = dict(
    x1="act", rstdA="pool", diag1="dve",
    x1t8="dve", x2a="dve", x2b="dve", rstdB="pool", x2t8="act",
    k2fm8_hi="dve", k2fm8_lo="dve", q2fm8="act", krm8="dve", w4t="dve",
    o2fm8="act", diag2="dve", x3_hi="dve", x3_lo="act", rstdC="pool",
    x3t8="act",
    ofin0="act", ofin1="act", rstdD="dve",
)

# psum / sbuf pool buffer counts (swept)
PSCFG = dict(psT=3, psACC=4, psTP=1, psTP2=1, psH=1, psY=4,
             psPJ=2, psRES=4, psBT=1, psO2=1, psH2=2, psZ=3, stats=6)


def build_nc():
    Alu = mybir.AluOpType
    Act = mybir.ActivationFunctionType
    nc = bacc.Bacc("TRN2", target_bir_lowering=False, debug=False,
                   num_devices=N_CORES)

    x_d = nc.dram_tensor("x", [V, P, D], BF16, kind="ExternalInput").ap()
    x8t_d = nc.dram_tensor("x8t", [V, D, P], FP8, kind="ExternalInput").ap()
    c4s_d = nc.dram_tensor("c4s", [128, 128], F32, kind="ExternalInput").ap()
    logm8_d = nc.dram_tensor("logm8", [128, 8], F32, kind="ExternalInput").ap()
    wpool_d = nc.dram_tensor("wpool", [P, S], BF16, kind="ExternalInput").ap()
    wq1_d = nc.dram_tensor("wq1", [D, D], BF16, kind="ExternalInput").ap()
    wk1t_d = nc.dram_tensor("wk1t", [D, D], BF16, kind="ExternalInput").ap()
    wo1t_d = nc.dram_tensor("wo1t", [D, D], BF16, kind="ExternalInput").ap()
    wq2t_d = nc.dram_tensor("wq2t", [D, D], FP8, kind="ExternalInput").ap()
    wk2t_d = nc.dram_tensor("wk2t", [D, D], FP8, kind="ExternalInput").ap()
    wo2t_d = nc.dram_tensor("wo2t", [D, D], FP8, kind="ExternalInput").ap()
    w1ft_d = nc.dram_tensor("w1ft", [D, H2], FP8, kind="ExternalInput").ap()
    w2ft_d = nc.dram_tensor("w2ft", [H2, D], FP8, kind="ExternalInput").ap()
    w3ft_d = nc.dram_tensor("w3ft", [D, H2], FP8, kind="ExternalInput").ap()
    w4ft_d = nc.dram_tensor("w4ft", [H2, D], FP8, kind="ExternalInput").ap()
    out_d = nc.dram_tensor("out", [V, P, D], F32, kind="ExternalOutput").ap()
    dbg = {}
    if DEBUG:
        dbg["attnT0"] = nc.dram_tensor("dbg_attnT0", [128, 256], BF16,
                                       kind="ExternalOutput").ap()
        dbg["mv1_0"] = nc.dram_tensor("dbg_mv1_0", [128, 2, 2], F32,
                                      kind="ExternalOutput").ap()
        dbg["rstd1_0"] = nc.dram_tensor("dbg_rstd1_0", [128, 2], F32,
                                        kind="ExternalOutput").ap()
        dbg["x1_0"] = nc.dram_tensor("dbg_x1_0", [128, 2, D], BF16,
                                     kind="ExternalOutput").ap()
        dbg["x2_0"] = nc.dram_tensor("dbg_x2_0", [128, 2, D], BF16,
                                     kind="ExternalOutput").ap()
        dbg["x3_0"] = nc.dram_tensor("dbg_x3_0", [128, 2, D], BF16,
                                     kind="ExternalOutput").ap()
        dbg["h8_0"] = nc.dram_tensor("dbg_h8_0", [128, 8, 256], FP8,
                                     kind="ExternalOutput").ap()
        dbg["krm0"] = nc.dram_tensor("dbg_krm0", [128, D], FP8,
                                     kind="ExternalOutput").ap()
        dbg["w4_0"] = nc.dram_tensor("dbg_w4_0", [128, 128], BF16,
                                     kind="ExternalOutput").ap()
        dbg["rs2_0"] = nc.dram_tensor("dbg_rs2_0", [128, 2], F32,
                                      kind="ExternalOutput").ap()
        dbg["k2fm0"] = nc.dram_tensor("dbg_k2fm0", [128, 4, 256], FP8,
                                      kind="ExternalOutput").ap()
        dbg["q2fm0"] = nc.dram_tensor("dbg_q2fm0", [128, 4, 256], FP8,
                                      kind="ExternalOutput").ap()
        dbg["o2fm0"] = nc.dram_tensor("dbg_o2fm0", [128, 4, 128], FP8,
                                      kind="ExternalOutput").ap()
    global _DBG
    _DBG = dbg

    with tile.TileContext(nc) as tc:
        _build_body(nc, tc, Alu, Act, x_d, x8t_d, c4s_d, logm8_d, wpool_d,
                    wq1_d, wk1t_d, wo1t_d, wq2t_d, wk2t_d, wo2t_d,
                    w1ft_d, w2ft_d, w3ft_d, w4ft_d, out_d)
    nc.compile()
    return nc


def _build_body(nc, tc, Alu, Act, x_d, x8t_d, c4s_d, logm8_d, wpool_d,
                wq1_d, wk1t_d, wo1t_d, wq2t_d, wk2t_d, wo2t_d,
                w1ft_d, w2ft_d, w3ft_d, w4ft_d, out_d):
    from contextlib import ExitStack

    DR = mybir.MatmulPerfMode.DoubleRow
    sw1 = WSCALE["w1ft"]
    sw2 = WSCALE["w2ft"]
    sw3 = WSCALE["w3ft"]
    sw4 = WSCALE["w4ft"]
    sq2 = WSCALE["wq2t"]
    sk2 = WSCALE["wk2t"]
    so2 = WSCALE["wo2t"]

    def eng(which):
        return {"act": nc.scalar, "dve": nc.vector, "pool": nc.gpsimd}[which]

    def cp(which, out, in_, scale=None):
        if which == "act":
            if scale is None:
                nc.scalar.copy(out=out, in_=in_)
            else:
                nc.scalar.activation(out=out, in_=in_, func=Act.Copy,
                                     scale=scale)
        else:
            e = eng(which)
            if scale is None:
                e.tensor_copy(out=out, in_=in_)
            else:
                e.tensor_scalar(out=out, in0=in_, scalar1=scale, scalar2=None,
                                op0=Alu.mult)

    def apply_ln(which, out, in_, sc, bi):
        if which == "act":
            nc.scalar.activation(out=out, in_=in_, func=Act.Identity,
                                 scale=sc, bias=bi)
        else:
            eng(which).tensor_scalar(out=out, in0=in_, scalar1=sc, scalar2=bi,
                                     op0=Alu.mult, op1=Alu.add)

    def load_wT(pool, dram_ap, din, dout, name, dt=BF16):
        t = pool.tile([128, din // 128, dout], dt, name=name)
        nc.sync.dma_start(out=t, in_=dram_ap.rearrange("(k p) d -> p k d", p=128))
        return t

    stack = ExitStack()
    with stack:
        persist = stack.enter_context(tc.tile_pool(name="persist", bufs=1))
        stats_p = stack.enter_context(tc.tile_pool(name="stats", bufs=PSCFG["stats"]))
        dram_p = stack.enter_context(tc.tile_pool(name="dramp", bufs=1, space="DRAM"))

        ident = persist.tile([128, 128], BF16, name="ident")
        make_identity(nc, ident)
        identf = persist.tile([128, 128], F32, name="identf")
        nc.gpsimd.tensor_copy(out=identf, in_=ident)
        ones = persist.tile([128, 1], BF16, name="ones")
        nc.vector.memset(ones, 1.0)
        eps_ap = persist.tile([128, 1], F32, name="eps_ap")
        nc.vector.memset(eps_ap, LN_EPS)
        c4s_sb = persist.tile([128, 128], F32, name="c4s_sb")
        nc.sync.dma_start(out=c4s_sb, in_=c4s_d)
        logm8 = persist.tile([128, 8], F32, name="logm8")
        nc.sync.dma_start(out=logm8, in_=logm8_d)
        # scaled identity injectors (residual via PSUM-accumulate)
        ident_w2 = persist.tile([128, 128], BF16, name="ident_w2")
        nc.gpsimd.tensor_scalar(out=ident_w2, in0=ident, scalar1=sw2,
                                scalar2=None, op0=Alu.mult)
        ident_w4 = persist.tile([128, 128], BF16, name="ident_w4")
        nc.gpsimd.tensor_scalar(out=ident_w4, in0=ident, scalar1=sw4,
                                scalar2=None, op0=Alu.mult)
        ident_lo = persist.tile([128, 128], BF16, name="ident_lo")
        nc.gpsimd.tensor_scalar(out=ident_lo, in0=ident, scalar1=SACT * so2,
                                scalar2=None, op0=Alu.mult)

        xc_dram = dram_p.tile([P, V, D], BF16, name="xc_dram")

        # x2T8_all: [128, kchunk=4, v=32, p=256] fp8, phase-2 feature-major
        x2t8p = stack.enter_context(tc.tile_pool(name="x2t8p", bufs=1))
        x2T8 = x2t8p.tile([128, 4, V, P], FP8, name="x2T8")

        def rstd_nr(which, mv, width, tag, inv_c=1.0, rs_ap=None):
            """mv [128, width, 2] = (mean, var) -> (rstd, negmr).

            The psum activations carry a known scale s: constant (1/s = inv_c)
            and/or a per-token tensor rs (pass rs_ap; total 1/s = inv_c/rs).
            Normalizes var to ~1, does quadratic rsqrt seed + one Newton
            step, then rescales.  LN eps is negligible at these variances.
            """
            e = eng(which)
            var = mv[:, :, 1]
            mean = mv[:, :, 0]
            if rs_ap is not None:
                inv = stats_p.tile([128, width], F32, tag=f"inv{tag}",
                                   name="inv")
                nc.vector.reciprocal(out=inv, in_=rs_ap)
                if inv_c != 1.0:
                    e.tensor_scalar(out=inv, in0=inv, scalar1=inv_c,
                                    scalar2=None, op0=Alu.mult)
                i2 = stats_p.tile([128, width], F32, tag=f"i2{tag}", name="i2")
                e.tensor_tensor(out=i2, in0=inv, in1=inv, op=Alu.mult)
                vn = stats_p.tile([128, width], F32, tag=f"vn{tag}", name="vn")
                e.tensor_tensor(out=vn, in0=var, in1=i2, op=Alu.mult)
            else:
                vn = stats_p.tile([128, width], F32, tag=f"vn{tag}", name="vn")
                e.tensor_scalar(out=vn, in0=var, scalar1=inv_c * inv_c,
                                scalar2=None, op0=Alu.mult)
            u = stats_p.tile([128, width], F32, tag=f"u{tag}", name="u")
            e.tensor_scalar(out=u, in0=vn, scalar1=LN_EPS - 1.0, scalar2=None,
                            op0=Alu.add)
            a = stats_p.tile([128, width], F32, tag=f"a{tag}", name="a")
            e.tensor_scalar(out=a, in0=u, scalar1=0.375, scalar2=-0.5,
                            op0=Alu.mult, op1=Alu.add)
            b = stats_p.tile([128, width], F32, tag=f"b{tag}", name="b")
            e.tensor_tensor(out=b, in0=u, in1=a, op=Alu.mult)
            y0 = stats_p.tile([128, width], F32, tag=f"y0{tag}", name="y0")
            e.tensor_scalar(out=y0, in0=b, scalar1=1.0, scalar2=None,
                            op0=Alu.add)
            q1 = stats_p.tile([128, width], F32, tag=f"q1{tag}", name="q1")
            e.tensor_tensor(out=q1, in0=y0, in1=y0, op=Alu.mult)
            q2 = stats_p.tile([128, width], F32, tag=f"q2{tag}", name="q2")
            rn = stats_p.tile([128, width], F32, tag=f"rn{tag}", name="rn")
            rstd = stats_p.tile([128, width], F32, tag=f"rstd{tag}", name="rstd")
            negmr = stats_p.tile([128, width], F32, tag=f"negmr{tag}",
                                 name="negmr")
            if which == "pool":
                # scalar_tensor_tensor is not a legal Pool opcode: expand
                t1 = stats_p.tile([128, width], F32, tag=f"t1{tag}", name="t1")
                e.tensor_scalar(out=t1, in0=q1, scalar1=-0.5, scalar2=None,
                                op0=Alu.mult)
                e.tensor_tensor(out=q2, in0=t1, in1=vn, op=Alu.mult)
                t2 = stats_p.tile([128, width], F32, tag=f"t2{tag}", name="t2")
                e.tensor_scalar(out=t2, in0=q2, scalar1=1.5, scalar2=None,
                                op0=Alu.add)
                e.tensor_tensor(out=rn, in0=t2, in1=y0, op=Alu.mult)
            else:
                e.scalar_tensor_tensor(out=q2, in0=q1, scalar=-0.5, in1=vn,
                                       op0=Alu.mult, op1=Alu.mult)
                e.scalar_tensor_tensor(out=rn, in0=q2, scalar=1.5, in1=y0,
                                       op0=Alu.add, op1=Alu.mult)
            # rescale: rstd = rstd_n / s
            if rs_ap is not None:
                e.tensor_tensor(out=rstd, in0=rn, in1=inv, op=Alu.mult)
            else:
                e.tensor_scalar(out=rstd, in0=rn, scalar1=inv_c, scalar2=None,
                                op0=Alu.mult)
            if which == "pool":
                t3 = stats_p.tile([128, width], F32, tag=f"t3{tag}", name="t3")
                e.tensor_scalar(out=t3, in0=mean, scalar1=-1.0, scalar2=None,
                                op0=Alu.mult)
                e.tensor_tensor(out=negmr, in0=t3, in1=rstd, op=Alu.mult)
            else:
                e.scalar_tensor_tensor(out=negmr, in0=mean, scalar=-1.0,
                                       in1=rstd, op0=Alu.mult, op1=Alu.mult)
            return rstd, negmr

        # ================= Phase 1 =================
        with ExitStack() as ph1:
            w1p = ph1.enter_context(tc.tile_pool(name="w1p", bufs=1))
            w1ft8 = load_wT(w1p, w1ft_d, D, H2, "w1ft8", FP8)
            w2ft8 = load_wT(w1p, w2ft_d, H2, D, "w2ft8", FP8)

            keyp = ph1.enter_context(tc.tile_pool(name="keyp", bufs=1))
            x1p = ph1.enter_context(tc.tile_pool(name="x1p", bufs=32))
            x1s = []

            with ExitStack() as sA:
                xbp = sA.enter_context(tc.tile_pool(name="xbp", bufs=6))
                xrp = sA.enter_context(tc.tile_pool(name="xrp", bufs=4))
                x8p = sA.enter_context(tc.tile_pool(name="x8p", bufs=4))
                work = sA.enter_context(tc.tile_pool(name="work1", bufs=2))
                sP2 = sA.enter_context(ExitStack())
                prep = sP2.enter_context(tc.tile_pool(name="prep", bufs=1))
                wpool_sb = prep.tile([128, 2, S], BF16, name="wpool_sb")
                nc.sync.dma_start(out=wpool_sb,
                                  in_=wpool_d.rearrange("(k p) s -> p k s",
                                                        p=128))
                wq1_sb = load_wT(prep, wq1_d, D, D, "wq1_sb")   # Wq1 as-is
                wk1t = load_wT(prep, wk1t_d, D, D, "wk1t")
                wo1t = load_wT(prep, wo1t_d, D, D, "wo1t")
                xp_ln = prep.tile([128, 4, D], BF16, name="xp_ln")
                xp_all = prep.tile([128, 4, D], F32, name="xp_all")

                # --- A: load x, pooled keys ---
                with ExitStack() as sPrep:
                    psA = sPrep.enter_context(tc.tile_pool(name="psA", bufs=2,
                                                           space="PSUM"))
                    for g in range(8):
                        xp_ps = psA.tile([16, 4, D], F32, tag="xp_ps",
                                         name="xp_ps")
                        for vi in range(4):
                            v = g * 4 + vi
                            xb = xbp.tile([128, 2, D], BF16, tag="big",
                                          name=f"xb{v}")
                            nc.sync.dma_start(
                                out=xb,
                                in_=x_d[v].rearrange("(c p) d -> p c d", p=128))
                            for pc in range(2):
                                nc.tensor.matmul(xp_ps[:, vi, :],
                                                 wpool_sb[:, pc, :],
                                                 xb[:, pc, :],
                                                 start=(pc == 0), stop=(pc == 1))
                        xp_sb = work.tile([16, 4, D], F32, tag="xp_sb",
                                          name="xp_sb")
                        nc.scalar.copy(out=xp_sb, in_=xp_ps)
                        for vi in range(4):
                            v = g * 4 + vi
                            p0 = 16 * (v % 8)
                            nc.sync.dma_start(
                                out=xp_all[p0:p0 + 16, v // 8, :],
                                in_=xp_sb[:, vi, :])

                # --- B: LN0 (var ~3e-3: keep exact sqrt, one table load) ---
                mv0 = stats_p.tile([128, 4, 2], F32, tag="mv0", name="mv0")
                for c in range(4):
                    st = work.tile([128, 6], F32, tag="st", name="st0")
                    nc.vector.bn_stats(out=st, in_=xp_all[:, c, :])
                    nc.vector.bn_aggr(out=mv0[:, c, :], in_=st)
                srt0 = stats_p.tile([128, 4], F32, tag="srt0", name="srt0")
                nc.scalar.activation(out=srt0, in_=mv0[:, :, 1],
                                     func=Act.Sqrt, bias=eps_ap)
                rstd0 = stats_p.tile([128, 4], F32, tag="rstd0", name="rstd0")
                nc.vector.reciprocal(out=rstd0, in_=srt0)
                negmr0 = stats_p.tile([128, 4], F32, tag="negmr0",
                                      name="negmr0")
                nc.vector.scalar_tensor_tensor(out=negmr0, in0=mv0[:, :, 0],
                                               scalar=-1.0, in1=rstd0,
                                               op0=Alu.mult, op1=Alu.mult)
                for c in range(4):
                    nc.scalar.activation(out=xp_ln[:, c, :],
                                         in_=xp_all[:, c, :],
                                         func=Act.Identity,
                                         scale=rstd0[:, c:c + 1],
                                         bias=negmr0[:, c:c + 1])

                # --- C: xpT (feature-major pooled keys) via DMA transpose ---
                xpT = prep.tile([128, 4, 512], BF16, name="xpT")
                for c in range(4):
                    nc.sync.dma_start_transpose(
                        out=xpT[:, :, c * 128:(c + 1) * 128], in_=xp_ln[:, c, :])

                # --- D: k_fm, M8 = Wq1^T k^T (fp8), N = k Wo1^T ---
                k_fm = prep.tile([128, 4, 512], BF16, name="k_fm")
                M8 = keyp.tile([128, 4, 512], FP8, name="M8")
                N_sm = keyp.tile([128, 4, 512], BF16, name="N_sm")
                with ExitStack() as sd:
                    psD = sd.enter_context(tc.tile_pool(name="psD", bufs=1,
                                                        space="PSUM"))
                    kf_ps = psD.tile([128, 4, D], F32, tag="dps", name="kf_ps")
                    for m in range(4):
                        for kc in range(4):
                            nc.tensor.matmul(kf_ps[:, m, :],
                                             wk1t[:, kc, m * 128:(m + 1) * 128],
                                             xpT[:, kc, :],
                                             start=(kc == 0), stop=(kc == 3))
                    nc.scalar.copy(out=k_fm, in_=kf_ps)
                    m_ps = psD.tile([128, 4, D], F32, tag="dps", name="m_ps")
                    for m in range(4):
                        for kc in range(4):
                            nc.tensor.matmul(m_ps[:, m, :],
                                             wq1_sb[:, kc, m * 128:(m + 1) * 128],
                                             k_fm[:, kc, :],
                                             start=(kc == 0), stop=(kc == 3))
                    nc.vector.tensor_scalar(out=M8, in0=m_ps, scalar1=SM,
                                            scalar2=None, op0=Alu.mult)
                    n_ps = psD.tile([128, 4, D], F32, tag="dps", name="n_ps")
                    for sc in range(4):
                        for kc in range(4):
                            nc.tensor.matmul(n_ps[:, sc, :],
                                             k_fm[:, kc, sc * 128:(sc + 1) * 128],
                                             wo1t[:, kc, :],
                                             start=(kc == 0), stop=(kc == 3))
                    nc.scalar.copy(out=N_sm, in_=n_ps)
                sP2.close()   # free prep tensors (xp_all/xpT/k_fm/w1 bf16)

                # ====== Loop A: patch attention (ACT: Exp only) ======
                # 3-stage software pipeline: each engine queue interleaves
                # iterations so in-order heads never block on cross-engine deps
                with ExitStack() as sa:
                    psT = sa.enter_context(tc.tile_pool(name="psT", bufs=PSCFG["psT"],
                                                        space="PSUM"))
                    psACC = sa.enter_context(tc.tile_pool(name="psACC", bufs=PSCFG["psACC"],
                                                          space="PSUM"))
                    stA = {}

                    def a_s0(v):
                        cg, r = v // 8, v % 8
                        xb = xrp.tile([128, 2, D], BF16, tag="xres",
                                      name=f"xr{v}")
                        nc.sync.dma_start(
                            out=xb,
                            in_=x_d[v].rearrange("(c p) d -> p c d", p=128))
                        x8 = x8p.tile([128, 4, P], FP8, tag="x8",
                                      name=f"x8_{v}")
                        nc.sync.dma_start(
                            out=x8,
                            in_=x8t_d[v].rearrange("(k p) t -> p k t", p=128))
                        # scores^T for the 8v slot group (junk rows killed by
                        # the exp log-mask bias); cols 256:258 hold rs sums
                        sT_ps = psT.tile([128, 512], F32, tag="sT", name="sT_ps")
                        for kk in range(2):
                            nc.tensor.matmul(
                                sT_ps[:, 0:256],
                                M8[:, 2 * kk:2 * kk + 2, cg * 128:(cg + 1) * 128],
                                x8[:, 2 * kk:2 * kk + 2, :],
                                start=(kk == 0), stop=(kk == 1), perf_mode=DR,
                                skip_group_check=True)
                        attnT = work.tile([128, 256], BF16, tag="attnT",
                                          name="attnT", bufs=3)
                        nc.scalar.activation(out=attnT, in_=sT_ps[:, 0:256],
                                             func=Act.Exp, scale=SCALE / SM,
                                             bias=logm8[:, r:r + 1])
                        for t in range(2):
                            nc.tensor.matmul(sT_ps[:, 256 + t:257 + t],
                                             attnT[:, t * 128:(t + 1) * 128],
                                             ones, start=False, stop=(t == 1),
                                             skip_group_check=True)
                        rs_sb = stats_p.tile([128, 2], F32, tag="rs_sb",
                                             name="rs_sb")
                        nc.vector.tensor_copy(out=rs_sb, in_=sT_ps[:, 256:258])
                        stA[v] = dict(xb=xb, attnT=attnT, rs=rs_sb)

                    def a_s1(v):
                        st = stA[v]
                        cg = v // 8
                        a1_pss = []
                        mv1 = stats_p.tile([128, 2, 2], F32, tag="mv1",
                                           name="mv1")
                        for t in range(2):
                            diag1 = work.tile([128, 128], BF16, tag="diag",
                                              name="diag1")
                            eng(ECH["diag1"]).tensor_scalar(
                                out=diag1, in0=ident,
                                scalar1=st["rs"][:, t:t + 1], scalar2=None,
                                op0=Alu.mult)
                            a1_ps = psACC.tile([128, D], F32, tag="acc",
                                               name="a1_ps")
                            nc.tensor.matmul(a1_ps, diag1, st["xb"][:, t, :],
                                             start=True, stop=False,
                                             skip_group_check=True)
                            nc.tensor.matmul(
                                a1_ps, st["attnT"][:, t * 128:(t + 1) * 128],
                                N_sm[:, cg, :], start=False, stop=True,
                                skip_group_check=True)
                            stt = work.tile([128, 6], F32, tag="st", name="st1")
                            nc.vector.bn_stats(out=stt, in_=a1_ps)
                            nc.vector.bn_aggr(out=mv1[:, t, :], in_=stt)
                            a1_pss.append(a1_ps)
                        st["a1"] = a1_pss
                        st["mv"] = mv1

                    def a_s2(v):
                        st = stA.pop(v)
                        rstd1, negmr1 = rstd_nr(ECH["rstdA"], st["mv"], 2, "1",
                                                rs_ap=st["rs"])
                        x1 = x1p.tile([128, 2, D], BF16, tag="x1",
                                      name=f"x1_{v}")
                        for t in range(2):
                            apply_ln(ECH["x1"], x1[:, t, :], st["a1"][t],
                                     rstd1[:, t:t + 1], negmr1[:, t:t + 1])
                        x1s.append(x1)

                    for i in range(V + 2):
                        if i < V:
                            a_s0(i)
                        if 1 <= i <= V:
                            a_s1(i - 1)
                        if i >= 2:
                            a_s2(i - 2)

            # ====== Loop B: FFN1 (ACT: Gelu only) ======
            with ExitStack() as sb:
                workb = sb.enter_context(tc.tile_pool(name="workb", bufs=2))
                psTP = sb.enter_context(tc.tile_pool(name="psTP", bufs=PSCFG["psTP"],
                                                     space="PSUM"))
                psH = sb.enter_context(tc.tile_pool(name="psH", bufs=PSCFG["psH"],
                                                    space="PSUM"))
                psY = sb.enter_context(tc.tile_pool(name="psY", bufs=PSCFG["psY"],
                                                    space="PSUM"))
                psTP2 = sb.enter_context(tc.tile_pool(name="psTP2", bufs=PSCFG["psTP2"],
                                                      space="PSUM"))
                stB = {}

                def b_s0(v):
                    x1 = x1s[v]
                    # x1^T (fp8) via PE transpose
                    tp_ps = psTP.tile([128, 4, 256], BF16, tag="tp",
                                      name="tp_ps")
                    for t in range(2):
                        for kc in range(4):
                            nc.tensor.matmul(
                                tp_ps[:, kc, t * 128:(t + 1) * 128],
                                x1[:, t, kc * 128:(kc + 1) * 128], ident,
                                is_transpose=True,
                                start=(t == 0 and kc == 0),
                                stop=(t == 1 and kc == 3),
                                skip_group_check=True)
                    x1T8 = workb.tile([128, 4, 256], FP8, tag="x1T8",
                                      name="x1T8", bufs=3)
                    cp(ECH["x1t8"], x1T8, tp_ps)
                    stB[v] = dict(x1T8=x1T8)

                def b_s1(v):
                    st = stB[v]
                    x1T8 = st["x1T8"]
                    h8 = workb.tile([128, 8, 256], FP8, tag="h8", name="h8",
                                    bufs=3)
                    for half in range(2):
                        h_ps = psH.tile([128, 4, 256], F32, tag="h",
                                        name="h_ps")
                        for hc in range(4):
                            col = (4 * half + hc) * 128
                            for kk in range(2):
                                nc.tensor.matmul(
                                    h_ps[:, hc, :],
                                    w1ft8[:, 2 * kk:2 * kk + 2, col:col + 128],
                                    x1T8[:, 2 * kk:2 * kk + 2, :],
                                    start=(hc % 2 == 0 and kk == 0),
                                    stop=(hc % 2 == 1 and kk == 1),
                                    perf_mode=DR, skip_group_check=True)
                        nc.scalar.activation(
                            out=h8[:, 4 * half:4 * half + 4, :],
                            in_=h_ps.rearrange("p a b -> p (a b)"),
                            func=Act.Gelu, scale=1.0 / sw1)
                    st["h8"] = h8

                def b_s2(v):
                    st = stB[v]
                    x1 = x1s[v]
                    h8 = st["h8"]
                    y_pss = []
                    mv2 = stats_p.tile([128, 2, 2], F32, tag="mv2", name="mv2")
                    for t in range(2):
                        y_ps = psY.tile([128, D], F32, tag="y", name="y_ps")
                        nc.tensor.matmul(y_ps, ident_w2, x1[:, t, :],
                                         start=True, stop=False,
                                         skip_group_check=True)
                        for hp in range(4):
                            nc.tensor.matmul(
                                y_ps,
                                h8[:, 2 * hp:2 * hp + 2, t * 128:(t + 1) * 128],
                                w2ft8[:, 2 * hp:2 * hp + 2, :],
                                start=False, stop=(hp == 3), perf_mode=DR,
                                skip_group_check=True)
                        stt = workb.tile([128, 6], F32, tag="st", name="st2")
                        nc.vector.bn_stats(out=stt, in_=y_ps)
                        nc.vector.bn_aggr(out=mv2[:, t, :], in_=stt)
                        y_pss.append(y_ps)
                    rstd2, negmr2 = rstd_nr(ECH["rstdB"], mv2, 2, "2",
                                            inv_c=1.0 / sw2)
                    st["y"] = y_pss
                    st["rstd"] = rstd2
                    st["negmr"] = negmr2

                def b_s3(v):
                    st = stB.pop(v)
                    x2_t = workb.tile([128, 2, D], BF16, tag="x2", name="x2_t",
                                      bufs=3)
                    for t in range(2):
                        apply_ln(ECH["x2a" if t == 0 else "x2b"],
                                 x2_t[:, t, :], st["y"][t],
                                 st["rstd"][:, t:t + 1],
                                 st["negmr"][:, t:t + 1])
                    # x2^T -> x2T8_all[:, :, v, :] (fp8 feature-major)
                    tp2 = psTP2.tile([128, 4, 256], BF16, tag="tp2",
                                     name="tp2")
                    for t in range(2):
                        for kc in range(4):
                            nc.tensor.matmul(
                                tp2[:, kc, t * 128:(t + 1) * 128],
                                x2_t[:, t, kc * 128:(kc + 1) * 128], ident,
                                is_transpose=True,
                                start=(t == 0 and kc == 0),
                                stop=(t == 1 and kc == 3),
                                skip_group_check=True)
                    cp(ECH["x2t8"], x2T8[:, :, v, :], tp2)
                    nc.sync.dma_start(
                        out=xc_dram[:, v, :].rearrange("(c p) d -> p c d",
                                                       p=128),
                        in_=x2_t)

                for i in range(V + 3):
                    if i < V:
                        b_s0(i)
                    if 1 <= i <= V:
                        b_s1(i - 1)
                    if 2 <= i <= V + 1:
                        b_s2(i - 2)
                    if i >= 3:
                        b_s3(i - 3)

        # ================= Phase 2 =================
        def xc_tile_ap(t):
            return bass.AP(tensor=xc_dram.tensor,
                           offset=xc_dram.offset + 4 * t * V * D,
                           ap=[[D, V], [V * D, 4], [1, D]])

        with ExitStack() as ph2:
            w2p = ph2.enter_context(tc.tile_pool(name="w2p", bufs=1))
            wq2t8 = load_wT(w2p, wq2t_d, D, D, "wq2t8", FP8)
            wk2t8 = load_wT(w2p, wk2t_d, D, D, "wk2t8", FP8)
            wo2t8 = load_wT(w2p, wo2t_d, D, D, "wo2t8", FP8)
            w3ft8 = load_wT(w2p, w3ft_d, D, H2, "w3ft8", FP8)
            w4ft8 = load_wT(w2p, w4ft_d, H2, D, "w4ft8", FP8)

            x3p = ph2.enter_context(tc.tile_pool(name="x3p", bufs=32))
            x3t8p = ph2.enter_context(tc.tile_pool(name="x3t8p", bufs=32))
            x3s, x3t8s = [], []

            # ====== Loop C: var attention + LN3 (ACT: Exp only) ======
            with ExitStack() as sc:
                workc = sc.enter_context(tc.tile_pool(name="workc", bufs=3))
                psPJ = sc.enter_context(tc.tile_pool(name="psPJ", bufs=PSCFG["psPJ"],
                                                     space="PSUM"))
                psRES = sc.enter_context(tc.tile_pool(name="psRES", bufs=PSCFG["psRES"],
                                                      space="PSUM"))
                psBT = sc.enter_context(tc.tile_pool(name="psBT", bufs=PSCFG["psBT"],
                                                     space="PSUM"))
                psO2 = sc.enter_context(tc.tile_pool(name="psO2", bufs=PSCFG["psO2"],
                                                     space="PSUM"))
                stC = {}

                def c_s0(pr):
                    t0 = 2 * pr
                    hi = t0 >= 32
                    xct = []
                    for ti in range(2):
                        xc_t = workc.tile([128, D], BF16, tag="xct",
                                          name=f"xct{ti}", bufs=6)
                        nc.sync.dma_start(out=xc_t, in_=xc_tile_ap(t0 + ti))
                        xct.append(xc_t)
                    # contiguous feature-major fp8 activations for this pair
                    # (tokens v-major per tile), gathered from x2T8 on Pool
                    xcT8 = workc.tile([128, 4, 256], FP8, tag="xcT8",
                                      name="xcT8", bufs=3)
                    nc.gpsimd.tensor_copy(
                        out=xcT8.rearrange("p c (t v q) -> p c v t q",
                                           t=2, v=V, q=4),
                        in_=x2T8[:, :, :, 4 * t0:4 * t0 + 8].rearrange(
                            "p c v (t q) -> p c v t q", t=2))
                    # k2 feature-major (fp8, scaled to SACT*k2)
                    k2fm8 = workc.tile([128, 4, 256], FP8, tag="k2fm8",
                                       name="k2fm8", bufs=3)
                    for mh in range(2):
                        pj = psPJ.tile([128, 512], F32, tag="proj", name="k2_ps")
                        for mi in range(2):
                            m = 2 * mh + mi
                            for kk in range(2):
                                nc.tensor.matmul(
                                    pj[:, mi * 256:(mi + 1) * 256],
                                    wk2t8[:, 2 * kk:2 * kk + 2,
                                          m * 128:(m + 1) * 128],
                                    xcT8[:, 2 * kk:2 * kk + 2, :],
                                    start=(mi == 0 and kk == 0),
                                    stop=(mi == 1 and kk == 1),
                                    perf_mode=DR, skip_group_check=True)
                        cp(ECH["k2fm8_hi" if hi else "k2fm8_lo"],
                           k2fm8[:, 2 * mh:2 * mh + 2, :].rearrange(
                               "p m t -> p (m t)"),
                           pj, scale=SACT / sk2)
                    st = stC[pr] = dict(xct=xct, k2fm8=k2fm8, hi=hi)

                    if hi:
                        q2fm8 = workc.tile([128, 4, 256], FP8, tag="q2fm8",
                                           name="q2fm8", bufs=3)
                        for mh in range(2):
                            pj = psPJ.tile([128, 512], F32, tag="proj",
                                           name="q2_ps")
                            for mi in range(2):
                                m = 2 * mh + mi
                                for kk in range(2):
                                    nc.tensor.matmul(
                                        pj[:, mi * 256:(mi + 1) * 256],
                                        wq2t8[:, 2 * kk:2 * kk + 2,
                                              m * 128:(m + 1) * 128],
                                        xcT8[:, 2 * kk:2 * kk + 2, :],
                                        start=(mi == 0 and kk == 0),
                                        stop=(mi == 1 and kk == 1),
                                        perf_mode=DR, skip_group_check=True)
                            cp(ECH["q2fm8"],
                               q2fm8[:, 2 * mh:2 * mh + 2, :].rearrange(
                                   "p m t -> p (m t)"),
                               pj, scale=SACT / sq2)
                        # k2 row-major (tokens on partitions) for o2fm
                        k2rm8 = []
                        for ti in range(2):
                            kr_ps = psPJ.tile([128, 512], F32, tag="proj",
                                              name="kr_ps")
                            for kk in range(2):
                                nc.tensor.matmul(
                                    kr_ps,
                                    xcT8[:, 2 * kk:2 * kk + 2,
                                         ti * 128:(ti + 1) * 128],
                                    wk2t8[:, 2 * kk:2 * kk + 2, :],
                                    start=(kk == 0), stop=(kk == 1),
                                    perf_mode=DR)
                            krm = workc.tile([128, D], FP8, tag="krm8",
                                             name=f"krm8_{ti}", bufs=4)
                            cp(ECH["krm8"], krm, kr_ps, scale=SACT / sk2)
                            k2rm8.append(krm)
                        st["q2fm8"] = q2fm8
                        st["k2rm8"] = k2rm8

                def c_s1(pr):
                    st = stC[pr]
                    hi = st["hi"]
                    xct = st["xct"]
                    k2fm8 = st["k2fm8"]
                    a2o_pss = []
                    mv3 = stats_p.tile([128, 2, 2], F32, tag="mv3", name="mv3")
                    rs2b = workc.tile([128, 2], F32, tag="rs2b", name="rs2b",
                                      bufs=3)
                    for ti in range(2):
                        a2o_ps = psRES.tile([128, D], F32, tag="res",
                                            name="a2o_ps")
                        if hi:
                            # scores TRANSPOSED: s4T[k, q] = k2^T q2 (+c4^T),
                            # so exp gives w4T directly (no PE transpose/copy);
                            # rs2[q] rides in psum col 128 via a ones-matmul
                            s4b = psPJ.tile([128, 512], F32, tag="proj",
                                            name="s4b")
                            s4_ps = s4b[:, 0:128]
                            for kk in range(2):
                                nc.tensor.matmul(
                                    s4_ps,
                                    k2fm8[:, 2 * kk:2 * kk + 2,
                                          ti * 128:(ti + 1) * 128],
                                    st["q2fm8"][:, 2 * kk:2 * kk + 2,
                                                ti * 128:(ti + 1) * 128],
                                    start=(kk == 0), stop=False, perf_mode=DR,
                                    skip_group_check=True)
                            # inject c4s (transposed log-count bias) in f32
                            nc.tensor.matmul(s4_ps, identf, c4s_sb,
                                             start=False, stop=True,
                                             skip_group_check=True)
                            rs2 = rs2b[:, ti:ti + 1]
                            w4T = workc.tile([128, 128], BF16, tag="w4Ts",
                                             name="w4T")
                            nc.scalar.activation(out=w4T, in_=s4_ps,
                                                 func=Act.Exp,
                                                 scale=SCALE / (SACT * SACT))
                            nc.tensor.matmul(s4b[:, 128:129], w4T, ones,
                                             start=False, stop=True,
                                             skip_group_check=True)
                            nc.vector.tensor_copy(out=rs2,
                                                  in_=s4b[:, 128:129])
                            o2_ps = psO2.tile([128, 4, 128], F32, tag="o2f",
                                              name="o2_ps")
                            for dc in range(4):
                                nc.tensor.matmul(
                                    o2_ps[:, dc, :],
                                    st["k2rm8"][ti][:, dc * 128:(dc + 1) * 128],
                                    w4T, start=(dc == 0), stop=(dc == 3),
                                    skip_group_check=True)
                            o2fm8 = workc.tile([128, 4, 128], FP8, tag="o2fm8",
                                               name="o2fm8")
                            cp(ECH["o2fm8"], o2fm8, o2_ps, scale=S_O2)
                            diag2 = workc.tile([128, 128], BF16, tag="diag2",
                                               name="diag2")
                            eng(ECH["diag2"]).tensor_scalar(
                                out=diag2, in0=ident, scalar1=rs2,
                                scalar2=so2 * S_O2 * SACT,
                                op0=Alu.mult, op1=Alu.mult)
                            nc.tensor.matmul(a2o_ps, diag2, xct[ti],
                                             start=True, stop=False,
                                             skip_group_check=True)
                            for kk in range(2):
                                nc.tensor.matmul(
                                    a2o_ps, o2fm8[:, 2 * kk:2 * kk + 2, :],
                                    wo2t8[:, 2 * kk:2 * kk + 2, :],
                                    start=False, stop=(kk == 1), perf_mode=DR,
                                    skip_group_check=True)
                        else:
                            nc.tensor.matmul(a2o_ps, ident_lo, xct[ti],
                                             start=True, stop=False,
                                             skip_group_check=True)
                            for kk in range(2):
                                nc.tensor.matmul(
                                    a2o_ps,
                                    k2fm8[:, 2 * kk:2 * kk + 2,
                                          ti * 128:(ti + 1) * 128],
                                    wo2t8[:, 2 * kk:2 * kk + 2, :],
                                    start=False, stop=(kk == 1), perf_mode=DR,
                                    skip_group_check=True)
                        stt = workc.tile([128, 6], F32, tag="st3", name="st3")
                        nc.vector.bn_stats(out=stt, in_=a2o_ps)
                        nc.vector.bn_aggr(out=mv3[:, ti, :], in_=stt)
                        a2o_pss.append(a2o_ps)
                    st["a2o"] = a2o_pss
                    st["mv"] = mv3
                    st["rs2b"] = rs2b

                def c_s2(pr):
                    st = stC.pop(pr)
                    if st["hi"]:
                        # a2o scale s = rs2 * (so2*S_O2*SACT) per token
                        rstd3, negmr3 = rstd_nr(
                            ECH["rstdC"], st["mv"], 2, "3",
                            inv_c=1.0 / (so2 * S_O2 * SACT), rs_ap=st["rs2b"])
                    else:
                        rstd3, negmr3 = rstd_nr(ECH["rstdC"], st["mv"], 2, "3",
                                                inv_c=1.0 / (SACT * so2))
                    x3 = x3p.tile([128, 2, D], BF16, tag="x3", name=f"x3_{pr}")
                    x3T8 = x3t8p.tile([128, 4, 256], FP8, tag="x3T8",
                                      name=f"x3T8_{pr}")
                    for ti in range(2):
                        apply_ln(ECH["x3_hi" if st["hi"] else "x3_lo"],
                                 x3[:, ti, :], st["a2o"][ti],
                                 rstd3[:, ti:ti + 1], negmr3[:, ti:ti + 1])
                        tp3 = psBT.tile([128, 8, 128], BF16, tag="bftp",
                                        name="tp3")
                        for kc in range(4):
                            nc.tensor.matmul(
                                tp3[:, kc, :],
                                x3[:, ti, kc * 128:(kc + 1) * 128],
                                ident, is_transpose=True,
                                start=(kc == 0), stop=(kc == 3),
                                skip_group_check=True)
                        cp(ECH["x3t8"], x3T8[:, :, ti * 128:(ti + 1) * 128],
                           tp3[:, 0:4, :])
                    x3s.append(x3)
                    x3t8s.append(x3T8)

                NPR = 32
                for i in range(NPR + 2):
                    if i < NPR:
                        c_s0(i)
                    if 1 <= i <= NPR:
                        c_s1(i - 1)
                    if i >= 2:
                        c_s2(i - 2)

            # ====== Loop D: FFN2 + LN4 (ACT: Gelu only) ======
            with ExitStack() as sd2:
                workd = sd2.enter_context(tc.tile_pool(name="workd", bufs=2))
                psH2 = sd2.enter_context(tc.tile_pool(name="psH2", bufs=PSCFG["psH2"],
                                                      space="PSUM"))
                psZ = sd2.enter_context(tc.tile_pool(name="psZ", bufs=PSCFG["psZ"],
                                                     space="PSUM"))
                stD = {}

                def d_s0(pr):
                    x3T8 = x3t8s[pr]
                    h28 = workd.tile([128, 8, 256], FP8, tag="h28", name="h28",
                                     bufs=3)
                    for half in range(2):
                        h2_ps = psH2.tile([128, 4, 256], F32, tag="h2",
                                          name="h2_ps")
                        for hc in range(4):
                            col = (4 * half + hc) * 128
                            for kk in range(2):
                                nc.tensor.matmul(
                                    h2_ps[:, hc, :],
                                    w3ft8[:, 2 * kk:2 * kk + 2, col:col + 128],
                                    x3T8[:, 2 * kk:2 * kk + 2, :],
                                    start=(hc % 2 == 0 and kk == 0),
                                    stop=(hc % 2 == 1 and kk == 1),
                                    perf_mode=DR, skip_group_check=True)
                        nc.scalar.activation(
                            out=h28[:, 4 * half:4 * half + 4, :],
                            in_=h2_ps.rearrange("p a b -> p (a b)"),
                            func=Act.Gelu, scale=1.0 / sw3)
                    stD[pr] = dict(h28=h28)

                def d_s1(pr):
                    st = stD[pr]
                    x3 = x3s[pr]
                    h28 = st["h28"]
                    mv4 = stats_p.tile([128, 2, 2], F32, tag="mv4", name="mv4")
                    z_pss = []
                    for ti in range(2):
                        z_ps = psZ.tile([128, D], F32, tag="z", name="z_ps")
                        nc.tensor.matmul(z_ps, ident_w4, x3[:, ti, :],
                                         start=True, stop=False,
                                         skip_group_check=True)
                        for hp in range(4):
                            nc.tensor.matmul(
                                z_ps,
                                h28[:, 2 * hp:2 * hp + 2,
                                    ti * 128:(ti + 1) * 128],
                                w4ft8[:, 2 * hp:2 * hp + 2, :],
                                start=False, stop=(hp == 3), perf_mode=DR,
                                skip_group_check=True)
                        stt = workd.tile([128, 6], F32, tag="st4", name="st4")
                        nc.vector.bn_stats(out=stt, in_=z_ps)
                        nc.vector.bn_aggr(out=mv4[:, ti, :], in_=stt)
                        z_pss.append(z_ps)
                    st["z"] = z_pss
                    rstd4, negmr4 = rstd_nr(ECH["rstdD"], mv4, 2, "4",
                                            inv_c=1.0 / sw4)
                    st["rstd"] = rstd4
                    st["negmr"] = negmr4

                def d_s2(pr):
                    st = stD.pop(pr)
                    t0 = 2 * pr
                    rstd4, negmr4 = st["rstd"], st["negmr"]
                    for ti in range(2):
                        t = t0 + ti
                        o = workd.tile([128, D], F32, tag="ofin", name="ofin")
                        apply_ln(ECH["ofin0" if ti == 0 else "ofin1"],
                                 o, st["z"][ti],
                                 rstd4[:, ti:ti + 1], negmr4[:, ti:ti + 1])
                        dst = bass.AP(tensor=out_d.tensor,
                                      offset=out_d.offset + 4 * t * D,
                                      ap=[[P * D, V], [D, 4], [1, D]])
                        nc.sync.dma_start(out=dst, in_=o)

                for i in range(34):
                    if i < 32:
                        d_s0(i)
                    if 1 <= i <= 32:
                        d_s1(i - 1)
                    if i >= 2:
                        d_s2(i - 2)


_NC_CACHE = None


def _get_nc():
    global _NC_CACHE
    if _NC_CACHE is None:
        _NC_CACHE = build_nc()
    return _NC_CACHE


def _pow2_scale(a):
    m = float(np.abs(a).max())
    if m <= 0:
        return 1.0
    return float(2.0 ** np.floor(np.log2(192.0 / m)))


def _prep_weights(inputs):
    bf = ml_dtypes.bfloat16
    e4 = ml_dtypes.float8_e4m3

    def tr(a):
        return np.ascontiguousarray(np.asarray(a, np.float32).T)

    raw = dict(
        w1ft=tr(inputs["W1f"]), w2ft=tr(inputs["W2f"]),
        w3ft=tr(inputs["W3f"]), w4ft=tr(inputs["W4f"]),
        wq2t=tr(inputs["Wq2"]), wk2t=tr(inputs["Wk2"]), wo2t=tr(inputs["Wo2"]),
    )
    if not WSCALE:
        for k, a in raw.items():
            WSCALE[k] = _pow2_scale(a)

    Wp = np.asarray(inputs["Wp"], np.float32)
    wpool = np.zeros((P, S), np.float32)
    for p in range(P):
        wpool[p, p % PERIOD] = Wp[0, p // PERIOD]

    w = dict(
        wpool=wpool.astype(bf),
        wq1=np.ascontiguousarray(np.asarray(inputs["Wq1"], np.float32)).astype(bf),
        wk1t=tr(inputs["Wk1"]).astype(bf),
        wo1t=tr(inputs["Wo1"]).astype(bf),
    )
    for k, a in raw.items():
        w[k] = (a * WSCALE[k]).astype(e4)

    logm8 = np.full((128, 8), -1e30, np.float32)
    for j in range(8):
        logm8[16 * j:16 * (j + 1), j] = 0.0
    w["logm8"] = logm8
    return w


def kernel(**inputs):
    w = _prep_weights(inputs)     # must run before build (sets WSCALE)
    nc = _get_nc()
    bf = ml_dtypes.bfloat16
    e4 = ml_dtypes.float8_e4m3
    x = np.asarray(inputs["x"], np.float32)
    ccc = np.asarray(inputs["var_ccc"])
    c4scale = (SACT * SACT) / SCALE
    in_maps = []
    for b in range(N_CORES):
        cnt = np.zeros((V, V), np.float32)
        for v in range(V):
            for n in range(N_REL):
                cnt[v, int(ccc[b, v, n])] += 1.0
        c4 = np.kron(cnt, np.eye(4, dtype=np.float32))
        c4 = np.where(c4 > 0, np.log(np.maximum(c4, 1e-9)), -1e30)
        # transposed: kernel computes scores as s4T[k, q]
        c4s = np.ascontiguousarray((c4 * c4scale).T).astype(np.float32)
        xb = np.ascontiguousarray(x[b])
        x8t = np.ascontiguousarray(np.swapaxes(xb, 1, 2)).astype(e4)
        in_maps.append({"x": xb.astype(bf), "x8t": x8t, "c4s": c4s, **w})
    res = run_bass_kernel_spmd(nc, in_maps, core_ids=list(range(N_CORES)))
    out = np.stack([res.results[b]["out"] for b in range(N_CORES)], axis=0)
    return out.astype(np.float32)


if __name__ == "__main__":
    rng = np.random.default_rng(0)
    fake = dict(
        x=rng.standard_normal((B, V, P, D), dtype=np.float32),
        var_ccc=rng.integers(0, V, (B, V, N_REL)),
        Wp=(rng.standard_normal((1, P // PERIOD)) * 0.02).astype(np.float32),
    )
    for nm in ["Wq1", "Wk1", "Wo1", "Wq2", "Wk2", "Wo2"]:
        fake[nm] = (rng.standard_normal((D, D)) * 0.02).astype(np.float32)
    fake["W1f"] = (rng.standard_normal((H2, D)) * 0.02).astype(np.float32)
    fake["W2f"] = (rng.standard_normal((D, H2)) * 0.02).astype(np.float32)
    fake["W3f"] = (rng.standard_normal((H2, D)) * 0.02).astype(np.float32)
    fake["W4f"] = (rng.standard_normal((D, H2)) * 0.02).astype(np.float32)
    o = kernel(**fake)
    print("out", o.shape, o.dtype, float(np.abs(o).max()))
